# revision 28
# baseline (speedup 1.0000x reference)
"""Distributed Trainium2 kernel for AM-normfree-softmax + MHE inter-class loss.

loss = CE(S*(emb @ normalize(W).T - M*onehot(y)), y)
       + sum_{i, j != y_i} 1/||w_hat_{y_i} - w_hat_j||^2 / (B*(C-1))

Strategy (classifier/tensor parallel, C sharded across 8 cores), v2:

Host: normalize W rows in f32, cast w_hat to fp8e4m3; per core ship the
shard transposed (wt [D, CPAD]) plus embT / wsT (= w_hat[y].T) in fp8.
Device: ONLY the two big matmuls, in fp8 DoubleRow mode (157 TF/s: each
instruction contracts a pair of 128-row K-blocks), streamed over 512-col
N-chunks into PSUM:
  - emb rows  -> ACT Exp (per-row bias from the first chunk's row max,
    accum_out) -> per-chunk exp sums (sslots)
  - ws rows   -> one fused DVE op per tile (CLAMP_RECIP_ACC_ANT,
    registered at import): accum += sum_j f(g_ij),
    f(g) = x/(x^2+lam), x = g-1.  Since ws rows are pre-normalized,
    sum_{j!=y} 1/||w_y-w_j||^2 = -1/2 sum_{j!=y} f(g_ij) with f == 1/x;
    the lam clamp bounds the j==y_i self-term (x ~ 0) by 1/(2*sqrt(lam))
    so no spike/mask matmul is needed at all.  Self/pad-column residues
    are subtracted exactly on the host.
No on-device collective: each core DMAs out a [128, 9] pack
(bias, expsum, inter-partial); the host does the cross-core logsumexp /
CE / inter merge in float64 (that's the gather/unshard step).
"""

from functools import lru_cache
from operator import add as _op_add

import ml_dtypes
import numpy as np

import concourse.bass as bass  # noqa: F401
import concourse.tile as tile
from concourse import bacc, mybir

# ---- custom fused DVE op: accum += sum_k f(x_k),
#   f(x) = 1 / min(x - s0, s1)   (s1 < 0: clamp toward the pole)
# For true terms (x - s0 <= -0.75) this is 1/(x - s0); the j == y_i
# self-term (x - s0 ~ 0) clamps to exactly s1, a bit-exact constant the
# host subtracts.  BITWISE_NOT exponent-flip seed (imm2 = -4/17) + one
# Newton step: 7 ALU stages + accumulate; ~0.35% max rel err.
import concourse.dve_ops as _dve_ops  # noqa: E402
from concourse.dve_spec import (  # noqa: E402
    AluOp as _DAluOp,
    Bin as _DBin,
    C0 as _DC0,
    C1 as _DC1,
    C2 as _DC2,
    Spec as _DSpec,
    Src0 as _DSrc0,
    Zero as _DZero,
    _has_src1 as _dve_has_src1,
    lower as _dve_lower,
)
from concourse.dve_uop import DveOpSpec as _DveOpSpec  # noqa: E402

_CRA_NAME = "CLAMP_RECIP_ACC_ANT"


def _cra_emulate(in0, s0, s1, imm2):
    x = (np.asarray(in0, dtype=np.float32) - np.float32(s0)).astype(np.float32)
    xc = np.minimum(x, np.float32(s1)).astype(np.float32)
    nd = (~xc.view(np.int32)).view(np.float32)
    y0 = (nd * np.float32(imm2)).astype(np.float32)
    t1 = (xc * y0).astype(np.float32)
    t2 = (np.float32(2.0) - t1).astype(np.float32)
    return (y0 * t2).astype(np.float32)


def _cra_reference(in0, in1, s0, s1, imm2):
    y = _cra_emulate(in0, s0, s1, imm2)
    return y, y.reshape(y.shape[0], -1).sum(axis=-1, keepdims=True)


def _register_cra():
    for op in _dve_ops.OPS:
        if op.name == _CRA_NAME:
            return op
    from concourse.dve_spec import minn as _dminn, One as _DOne
    xc = _dminn(_DSrc0 - _DC0, _DC1)
    nd = _DBin(_DAluOp.BITWISE_NOT, xc, xc)
    y0 = nd * _DC2
    body = y0 * ((_DOne + _DOne) - (xc * y0))
    spec = _DSpec(body=body, accum=_op_add, accum_init=_DZero,
                  reference=_cra_reference)
    row = max(_dve_ops._SUB_OPCODE_FOR_NAME.values()) + 1
    assert row < 0x20
    _dve_ops._SUB_OPCODE_FOR_NAME[_CRA_NAME] = row
    shas = {}
    for ver in ("v3", "v4"):
        tmp = _DveOpSpec(name=_CRA_NAME, opcode=row,
                         uops=_dve_lower(spec, ver=ver),
                         rd1_en=_dve_has_src1(spec))
        shas[ver] = tmp.sha(ver)
    op = _dve_ops.DveOp(_CRA_NAME, spec, subdim=False, uops_sha=shas)
    _dve_ops.OPS.append(op)
    _dve_ops.CUSTOM_DVE_SPECS[_CRA_NAME] = spec
    return op


_CRA_OP = _register_cra()
_CRA_SEED = -4.0 / 17.0
CLAMP = -0.02

F32 = mybir.dt.float32
BF16 = mybir.dt.bfloat16
FP8 = mybir.dt.float8e4
AX = mybir.AxisListType
ALU = mybir.AluOpType
ACTF = mybir.ActivationFunctionType
DR = mybir.MatmulPerfMode.DoubleRow
FP8NP = ml_dtypes.float8_e4m3fn

B, D, C = 512, 512, 50000
NCORES = 8
CSH = C // NCORES          # 6250 classes per core
CPAD = 6272                # 49 * 128, padded shard width
NPAD = CPAD - CSH          # 22 zero pad columns
S_SCALE = 30.0
MARGIN = 0.2
LMD = 1.0
SLACK = 46.0               # exp-bias undershoot headroom (in logit units)

KB = D // 128              # 4 contraction blocks -> 2 DoubleRow pairs
MT = B // 128              # 4 M-tiles per operand group
# 6 x 1024-col units (2 PSUM banks each, epilogue runs once per unit)
# then the 128-col remainder; per-row exp bias comes from unit 0's row max
UNITS = [(j * 1024, 1024) for j in range(6)] + [(6144, 128)]
NCHUNK = len(UNITS)


def _build_graph():
    nc = bacc.Bacc("TRN2", target_bir_lowering=False, debug=False,
                   num_devices=NCORES)

    # all inputs arrive pre-arranged in SBUF per-partition layout so every
    # DMA moves >=2KB contiguous lines (512B lines run at ~70GB/s, 4KB at
    # ~300GB/s): wt[p, u-major (kb, col)] etc.
    wt = nc.declare_dram_parameter("wt", [128, KB * CPAD], FP8, isOutput=False)
    embT = nc.declare_dram_parameter("embt", [128, KB * B], FP8,
                                     isOutput=False)
    wsT = nc.declare_dram_parameter("wst", [128, KB * B], FP8, isOutput=False)
    out_p = nc.declare_dram_parameter("out", [128, 9], F32, isOutput=True)

    with tile.TileContext(nc) as tc:
        with (
            tc.tile_pool(name="consts", bufs=1) as consts,
            tc.tile_pool(name="stat", bufs=1) as statp,
            tc.tile_pool(name="pers", bufs=1) as pers,
            tc.tile_pool(name="escr", bufs=4) as escr_p,
            tc.tile_pool(name="rscr", bufs=4) as rscr_p,
            tc.tile_pool(name="mrg", bufs=1) as mrg_p,
            tc.tile_pool(name="pse", bufs=2, space="PSUM") as pse_p,
            tc.tile_pool(name="psw", bufs=2, space="PSUM") as psw_p,
        ):
            # ---- inputs: one DMA per unit block (4KB contiguous lines),
            # spread over three queues, first-needed first ----
            embT_sb = statp.tile([128, KB, B], FP8)
            wsT_sb = statp.tile([128, KB, B], FP8)
            wt_u = [statp.tile([128, KB, w], FP8, name=f"wt{u}")
                    for u, (_, w) in enumerate(UNITS)]
            uoff = [KB * c0 for c0, _ in UNITS]

            def _ldu(q, u, kp):     # one kb-pair half of a unit block
                w = UNITS[u][1]
                off = uoff[u] + 2 * kp * w
                q.dma_start(
                    out=wt_u[u][:, 2 * kp:2 * kp + 2, :],
                    in_=wt[:, off:off + 2 * w].rearrange(
                        "p (k c) -> p k c", k=2))

            # priority order: embT + unit0 first (gates the first matmul),
            # then units in consumption order; aggregate HBM bw is the limit
            _ldu(nc.sync, 0, 0)
            _ldu(nc.gpsimd, 0, 1)
            nc.scalar.dma_start(out=embT_sb[:, :, :],
                                in_=embT[:, :].rearrange("p (k c) -> p k c",
                                                         k=KB))
            nc.gpsimd.dma_start(out=wsT_sb[:, :, :],
                                in_=wsT[:, :].rearrange("p (k c) -> p k c",
                                                        k=KB))
            nc.sync.dma_start(
                out=wt_u[6][:, :, :],
                in_=wt[:, uoff[6]:uoff[6] + KB * 128].rearrange(
                    "p (k c) -> p k c", k=KB))
            _ldu(nc.scalar, 4, 0)
            _ldu(nc.scalar, 4, 1)
            for u in (1, 2, 3):
                _ldu(nc.sync, u, 0)
                _ldu(nc.gpsimd, u, 1)
            _ldu(nc.scalar, 5, 0)
            _ldu(nc.scalar, 5, 1)

            # dummy activation traced after the DMA issues: pulls the
            # one-time ACT Exp table load off the first tile's critical path
            warm_t = consts.tile([1, 1], F32)
            nc.vector.memset(warm_t, 1.0)
            warm_o = consts.tile([1, 1], F32)
            nc.scalar.activation(warm_o, warm_t, ACTF.Exp)

            # ---- persistent accumulators ----
            bias_t = pers.tile([128, MT], F32)          # per-row exp bias
            sslots = pers.tile([128, MT, NCHUNK], F32)  # per-chunk exp sums
            islots = pers.tile([128, MT, NCHUNK], F32)  # per-chunk f-sums

            # ---- main loop: units outer, m inner; per (m, unit) the K=512
            # contraction is 2 DoubleRow pairs x (up to) 2 column sub-blocks,
            # all into one bank-aligned [128, 1024] PSUM tile.  The small
            # 128-col remainder unit runs second so its epilogue-bound
            # matmuls hide mid-stream instead of stalling the tail. ----
            M_FIRST = list(range(2 * MT))            # emb m's before wsT lands
            M_STEADY = [0, 4, 1, 5, 2, 6, 3, 7]      # even ACT/DVE arrival
            for u in (0, 1, 2, 3, 4, 5, 6):
                c0, nco = UNITS[u]
                subs = [(so, min(512, nco - so)) for so in range(0, nco, 512)]
                for m in (M_FIRST if u == 0 else M_STEADY):
                    is_ws = m >= MT
                    mm = m % MT
                    stat = wsT_sb if is_ws else embT_sb
                    ps = (psw_p if is_ws else pse_p).tile(
                        [128, 1024], F32, tag="mm", name=f"ps{m}u{u}")
                    for kp in range(2):
                        lhsT = stat[:, 2 * kp:2 * kp + 2,
                                    mm * 128:(mm + 1) * 128]
                        for so, sw in subs:
                            nc.tensor.matmul(
                                ps[:, so:so + sw], lhsT,
                                wt_u[u][:, 2 * kp:2 * kp + 2,
                                        so:so + sw],
                                start=(kp == 0), stop=(kp == 1),
                                perf_mode=DR)
                    if not is_ws:
                        if u == 0:
                            mx = mrg_p.tile([128, 1], F32, tag="mx",
                                            name=f"mx{mm}")
                            nc.vector.reduce_max(mx, ps[:, :nco], axis=AX.X)
                            nc.vector.tensor_scalar(
                                out=bias_t[:, mm:mm + 1], in0=mx,
                                scalar1=-S_SCALE, scalar2=-SLACK,
                                op0=ALU.mult, op1=ALU.add)
                        es = escr_p.tile([128, 1024], BF16, tag="es")
                        nc.scalar.activation(
                            es[:, :nco], ps[:, :nco], ACTF.Exp,
                            bias=bias_t[:, mm:mm + 1], scale=S_SCALE,
                            accum_out=sslots[:, mm, u:u + 1])
                    else:
                        rr = rscr_p.tile([128, 1024], BF16, tag="rr")
                        nc.vector._custom_dve(
                            _CRA_OP, out=rr[:, :nco], in0=ps[:, :nco],
                            s0=1.0, s1=CLAMP, imm2=_CRA_SEED,
                            accum_out=islots[:, mm, u:u + 1])

            # ---- pack per-core partials and DMA out; host merges ----
            pack = mrg_p.tile([128, 9], F32)
            nc.vector.tensor_copy(out=pack[:, 0:MT], in_=bias_t)
            for m in range(MT):
                nc.vector.reduce_sum(pack[:, MT + m:MT + m + 1],
                                     sslots[:, m, :], axis=AX.X)
            iview = islots[:, :, :].rearrange("p m c -> p (m c)")
            nc.vector.reduce_sum(pack[:, 8:9], iview, axis=AX.X)
            nc.sync.dma_start(out=out_p[:, :], in_=pack[:, :])

    nc.compile()
    return nc


@lru_cache(maxsize=2)
def _graph_cached():
    return _build_graph()


def _host_prep(emb, W, y):
    emb = np.ascontiguousarray(np.asarray(emb), dtype=np.float32)
    W = np.ascontiguousarray(np.asarray(W), dtype=np.float32)
    y = np.asarray(y).astype(np.int64)

    norms = np.sqrt(np.einsum("cd,cd->c", W, W, dtype=np.float64))
    What = (W / norms[:, None].astype(np.float32)).astype(np.float32)
    What8 = What.astype(FP8NP)                      # (C, D) fp8
    emb8 = emb.astype(FP8NP)                        # (B, D) fp8
    ws8 = What8[y]                                  # (B, D) fp8

    def _p_kc(xT):      # (D, ncol) -> (128, KB*ncol) SBUF layout
        return np.ascontiguousarray(
            xT.reshape(KB, 128, -1).transpose(1, 0, 2).reshape(128, -1))

    embT8 = _p_kc(emb8.T)
    wsT8 = _p_kc(ws8.T)

    in_maps = []
    for c in range(NCORES):
        wt_c = np.zeros((D, CPAD), dtype=FP8NP)
        wt_c[:, :CSH] = What8[c * CSH:(c + 1) * CSH].T
        # unit-major blocks, each [128, KB*w] contiguous per partition
        blk = wt_c.reshape(KB, 128, CPAD)
        host = np.concatenate(
            [np.ascontiguousarray(blk[:, :, c0:c0 + w].transpose(1, 0, 2)
                                  ).reshape(128, KB * w)
             for c0, w in UNITS], axis=1)
        in_maps.append({"wt": host, "embt": embT8, "wst": wsT8})
    return in_maps, emb, What, What8, emb8, ws8, y


def _host_merge(packs, emb, What, What8, emb8, ws8, y):
    """Cross-core merge in f64: logsumexp for CE, corrected sum for inter."""
    # pack[p, 0:4]=bias, [4:8]=expsum (row index = m*128+p), [8]=inter
    bias = np.stack([p[:, 0:MT].T.reshape(B) for p in packs])    # (8, B)
    ssum = np.stack([p[:, MT:2 * MT].T.reshape(B) for p in packs])
    nb = -bias.astype(np.float64)           # sum_j e^{l_ij} = s_ic * e^{-b_ic}
    s64 = np.maximum(ssum.astype(np.float64), 1e-300)
    mx = nb.max(axis=0)
    stot = (s64 * np.exp(nb - mx[None, :])).sum(axis=0)
    lse = np.log(stot) + mx                                      # (B,)

    # exact target logit in f64 from the f32-normalized weights
    cos_y = np.einsum("bd,bd->b", emb.astype(np.float64),
                      What[y].astype(np.float64))
    tgt = S_SCALE * (cos_y - MARGIN)
    ce = float(np.mean(lse - tgt))

    inter_raw = float(sum(float(p[:, 8].sum()) for p in packs))
    # subtract the self-term (j == y_i) residues: x = ||w_hat_fp8||^2 - 1
    # clamps to exactly s1 on device; the emulation applies the same min
    n2 = np.einsum("bd,bd->b", ws8.astype(np.float32),
                   ws8.astype(np.float32))
    inter_raw -= float(
        _cra_emulate(n2, 1.0, CLAMP, _CRA_SEED).astype(np.float64).sum())
    # subtract the pad-column residues: g = 0 exactly, NPAD cols per core
    fpad = float(_cra_emulate(np.zeros((1,), np.float32), 1.0, CLAMP,
                              _CRA_SEED)[0])
    inter_raw -= NCORES * B * NPAD * fpad
    inter = -0.5 * inter_raw / (B * (C - 1.0))

    return np.float32(ce + LMD * inter)


def run(emb, W, y, trace=False):
    from concourse.bass_utils import run_bass_kernel_spmd

    in_maps, emb_f, What, What8, emb8, ws8, y64 = _host_prep(emb, W, y)
    nc = _graph_cached()
    res = run_bass_kernel_spmd(nc, in_maps, core_ids=list(range(NCORES)),
                               trace=trace)
    packs = [np.asarray(res.results[c]["out"], dtype=np.float32)
             for c in range(NCORES)]
    val = _host_merge(packs, emb_f, What, What8, emb8, ws8, y64)
    return val, res


def kernel(emb, W, y):
    val, _ = run(emb, W, y, trace=False)
    return val


if __name__ == "__main__":
    rng = np.random.default_rng(0)
    emb = rng.standard_normal((B, D)).astype(np.float32)
    W = rng.standard_normal((C, D)).astype(np.float32)
    y = rng.integers(0, C, size=(B,)).astype(np.int64)
    print("loss:", kernel(emb, W, y))


# revision 36
# speedup vs baseline: 1.0384x; 1.0384x over previous
"""Distributed Trainium2 kernel for AM-normfree-softmax + MHE inter-class loss.

loss = CE(S*(emb @ normalize(W).T - M*onehot(y)), y)
       + sum_{i, j != y_i} 1/||w_hat_{y_i} - w_hat_j||^2 / (B*(C-1))

Strategy (classifier/tensor parallel, C sharded across 8 cores), v2:

Host: normalize W rows in f32, cast w_hat to fp8e4m3; per core ship the
shard transposed (wt [D, CPAD]) plus embT / wsT (= w_hat[y].T) in fp8.
Device: ONLY the two big matmuls, in fp8 DoubleRow mode (157 TF/s: each
instruction contracts a pair of 128-row K-blocks), streamed over 512-col
N-chunks into PSUM:
  - emb rows  -> ACT Exp (per-row bias from the first chunk's row max,
    accum_out) -> per-chunk exp sums (sslots)
  - ws rows   -> one fused DVE op per tile (CLAMP_RECIP_ACC_ANT,
    registered at import): accum += sum_j f(g_ij),
    f(g) = x/(x^2+lam), x = g-1.  Since ws rows are pre-normalized,
    sum_{j!=y} 1/||w_y-w_j||^2 = -1/2 sum_{j!=y} f(g_ij) with f == 1/x;
    the lam clamp bounds the j==y_i self-term (x ~ 0) by 1/(2*sqrt(lam))
    so no spike/mask matmul is needed at all.  Self/pad-column residues
    are subtracted exactly on the host.
No on-device collective: each core DMAs out a [128, 9] pack
(bias, expsum, inter-partial); the host does the cross-core logsumexp /
CE / inter merge in float64 (that's the gather/unshard step).
"""

from functools import lru_cache
from operator import add as _op_add

import ml_dtypes
import numpy as np

import concourse.bass as bass  # noqa: F401
import concourse.tile as tile
from concourse import bacc, mybir

# ---- custom fused DVE op: accum += sum_k f(x_k),
#   f(x) = 1 / min(x - s0, s1)   (s1 < 0: clamp toward the pole)
# For true terms (x - s0 <= -0.75) this is 1/(x - s0); the j == y_i
# self-term (x - s0 ~ 0) clamps to exactly s1, a bit-exact constant the
# host subtracts.  BITWISE_NOT exponent-flip seed (imm2 = -4/17) + one
# Newton step: 7 ALU stages + accumulate; ~0.35% max rel err.
import concourse.dve_ops as _dve_ops  # noqa: E402
from concourse.dve_spec import (  # noqa: E402
    AluOp as _DAluOp,
    Bin as _DBin,
    C0 as _DC0,
    C1 as _DC1,
    C2 as _DC2,
    Spec as _DSpec,
    Src0 as _DSrc0,
    Zero as _DZero,
    _has_src1 as _dve_has_src1,
    lower as _dve_lower,
)
from concourse.dve_uop import DveOpSpec as _DveOpSpec  # noqa: E402

_CRA_NAME = "CLAMP_RECIP_ACC_ANT"


def _cra_emulate(in0, s0, s1, imm2):
    x = (np.asarray(in0, dtype=np.float32) - np.float32(s0)).astype(np.float32)
    xc = np.minimum(x, np.float32(s1)).astype(np.float32)
    nd = (~xc.view(np.int32)).view(np.float32)
    y0 = (nd * np.float32(imm2)).astype(np.float32)
    t1 = (xc * y0).astype(np.float32)
    t2 = (np.float32(2.0) - t1).astype(np.float32)
    return (y0 * t2).astype(np.float32)


def _cra_reference(in0, in1, s0, s1, imm2):
    y = _cra_emulate(in0, s0, s1, imm2)
    return y, y.reshape(y.shape[0], -1).sum(axis=-1, keepdims=True)


def _register_cra():
    for op in _dve_ops.OPS:
        if op.name == _CRA_NAME:
            return op
    from concourse.dve_spec import minn as _dminn, One as _DOne
    xc = _dminn(_DSrc0 - _DC0, _DC1)
    nd = _DBin(_DAluOp.BITWISE_NOT, xc, xc)
    y0 = nd * _DC2
    body = y0 * ((_DOne + _DOne) - (xc * y0))
    spec = _DSpec(body=body, accum=_op_add, accum_init=_DZero,
                  reference=_cra_reference)
    row = max(_dve_ops._SUB_OPCODE_FOR_NAME.values()) + 1
    assert row < 0x20
    _dve_ops._SUB_OPCODE_FOR_NAME[_CRA_NAME] = row
    shas = {}
    for ver in ("v3", "v4"):
        tmp = _DveOpSpec(name=_CRA_NAME, opcode=row,
                         uops=_dve_lower(spec, ver=ver),
                         rd1_en=_dve_has_src1(spec))
        shas[ver] = tmp.sha(ver)
    op = _dve_ops.DveOp(_CRA_NAME, spec, subdim=False, uops_sha=shas)
    _dve_ops.OPS.append(op)
    _dve_ops.CUSTOM_DVE_SPECS[_CRA_NAME] = spec
    return op


_CRA_OP = _register_cra()
_CRA_SEED = -4.0 / 17.0
CLAMP = -0.02

F32 = mybir.dt.float32
BF16 = mybir.dt.bfloat16
FP8 = mybir.dt.float8e4
AX = mybir.AxisListType
ALU = mybir.AluOpType
ACTF = mybir.ActivationFunctionType
DR = mybir.MatmulPerfMode.DoubleRow
FP8NP = ml_dtypes.float8_e4m3fn

B, D, C = 512, 512, 50000
NCORES = 8
CSH = C // NCORES          # 6250 classes per core
S_SCALE = 30.0
MARGIN = 0.2
LMD = 1.0
SLACK = 46.0               # exp-bias undershoot headroom (in logit units)

KB = D // 128              # 4 contraction blocks -> 2 DoubleRow pairs
MT = B // 128              # 4 M-tiles per operand group
# 6 x 1024-col units (2 PSUM banks each, epilogue runs once per unit)
# then the 106-col remainder; per-row exp bias comes from unit 0's row max
UNITS = [(j * 1024, 1024) for j in range(6)] + [(6144, CSH - 6144)]
NCHUNK = len(UNITS)


def _build_graph():
    nc = bacc.Bacc("TRN2", target_bir_lowering=False, debug=False,
                   num_devices=NCORES)

    # all inputs arrive pre-arranged in SBUF per-partition layout so every
    # DMA moves >=2KB contiguous lines (512B lines run at ~70GB/s, 4KB at
    # ~300GB/s): wt[p, u-major (kb, col)] etc.
    wt = nc.declare_dram_parameter("wt", [128, KB * CSH], FP8, isOutput=False)
    embT = nc.declare_dram_parameter("embt", [128, KB * B], FP8,
                                     isOutput=False)
    wsT = nc.declare_dram_parameter("wst", [128, KB * B], FP8, isOutput=False)
    out_p = nc.declare_dram_parameter("out", [128, 9], F32, isOutput=True)

    with tile.TileContext(nc) as tc:
        with (
            tc.tile_pool(name="consts", bufs=1) as consts,
            tc.tile_pool(name="stat", bufs=1) as statp,
            tc.tile_pool(name="pers", bufs=1) as pers,
            tc.tile_pool(name="escr", bufs=4) as escr_p,
            tc.tile_pool(name="rscr", bufs=3) as rscr_p,
            tc.tile_pool(name="mrg", bufs=1) as mrg_p,
            tc.tile_pool(name="ps", bufs=4, space="PSUM") as ps_p,
        ):
            # ---- inputs: one DMA per unit block (4KB contiguous lines),
            # spread over three queues, first-needed first ----
            embT_sb = statp.tile([128, KB, B], FP8)
            wsT_sb = statp.tile([128, KB, B], FP8)
            wt_u = [statp.tile([128, KB, w], FP8, name=f"wt{u}")
                    for u, (_, w) in enumerate(UNITS)]
            uoff = [KB * c0 for c0, _ in UNITS]

            def _ldu(q, u, kp):     # one kb-pair half of a unit block
                w = UNITS[u][1]
                off = uoff[u] + 2 * kp * w
                q.dma_start(
                    out=wt_u[u][:, 2 * kp:2 * kp + 2, :],
                    in_=wt[:, off:off + 2 * w].rearrange(
                        "p (k c) -> p k c", k=2))

            # priority order: embT + unit0 first (gates the first matmul),
            # then units in consumption order; aggregate HBM bw is the limit
            _ldu(nc.sync, 0, 0)
            _ldu(nc.gpsimd, 0, 1)
            nc.scalar.dma_start(out=embT_sb[:, :, :],
                                in_=embT[:, :].rearrange("p (k c) -> p k c",
                                                         k=KB))
            nc.gpsimd.dma_start(out=wsT_sb[:, :, :],
                                in_=wsT[:, :].rearrange("p (k c) -> p k c",
                                                        k=KB))
            nc.sync.dma_start(
                out=wt_u[6][:, :, :],
                in_=wt[:, uoff[6]:].rearrange("p (k c) -> p k c", k=KB))
            _ldu(nc.scalar, 4, 0)
            _ldu(nc.scalar, 4, 1)
            for u in (1, 2, 3):
                _ldu(nc.sync, u, 0)
                _ldu(nc.gpsimd, u, 1)
            _ldu(nc.scalar, 5, 0)
            _ldu(nc.scalar, 5, 1)

            # dummy activation traced after the DMA issues: pulls the
            # one-time ACT Exp table load off the first tile's critical path
            warm_t = consts.tile([1, 1], F32)
            nc.vector.memset(warm_t, 1.0)
            warm_o = consts.tile([1, 1], F32)
            nc.scalar.activation(warm_o, warm_t, ACTF.Exp)

            # ---- persistent accumulators ----
            bias_t = pers.tile([128, MT], F32)          # per-row exp bias
            sslots = pers.tile([128, MT, NCHUNK], F32)  # per-chunk exp sums
            islots = pers.tile([128, MT, NCHUNK], F32)  # per-chunk f-sums

            # ---- main loop: units outer, m inner; per (m, unit) the K=512
            # contraction is 2 DoubleRow pairs x (up to) 2 column sub-blocks,
            # all into one bank-aligned [128, 1024] PSUM tile.  The small
            # 128-col remainder unit runs second so its epilogue-bound
            # matmuls hide mid-stream instead of stalling the tail. ----
            M_FIRST = list(range(2 * MT))            # emb m's before wsT lands
            M_STEADY = [0, 4, 1, 5, 2, 6, 3, 7]      # even ACT/DVE arrival
            for u in (0, 1, 2, 3, 4, 5, 6):
                c0, nco = UNITS[u]
                subs = [(so, min(512, nco - so)) for so in range(0, nco, 512)]
                for m in (M_FIRST if u == 0 else M_STEADY):
                    is_ws = m >= MT
                    mm = m % MT
                    stat = wsT_sb if is_ws else embT_sb
                    ps = ps_p.tile([128, 1024], F32, tag="mm",
                                   name=f"ps{m}u{u}")
                    for kp in range(2):
                        lhsT = stat[:, 2 * kp:2 * kp + 2,
                                    mm * 128:(mm + 1) * 128]
                        for so, sw in subs:
                            nc.tensor.matmul(
                                ps[:, so:so + sw], lhsT,
                                wt_u[u][:, 2 * kp:2 * kp + 2,
                                        so:so + sw],
                                start=(kp == 0), stop=(kp == 1),
                                perf_mode=DR)
                    if not is_ws:
                        if u == 0:
                            mx = mrg_p.tile([128, 1], F32, tag="mx",
                                            name=f"mx{mm}")
                            nc.vector.reduce_max(mx, ps[:, :nco], axis=AX.X)
                            nc.vector.tensor_scalar(
                                out=bias_t[:, mm:mm + 1], in0=mx,
                                scalar1=-S_SCALE, scalar2=-SLACK,
                                op0=ALU.mult, op1=ALU.add)
                        es = escr_p.tile([128, 1024], BF16, tag="es")
                        nc.scalar.activation(
                            es[:, :nco], ps[:, :nco], ACTF.Exp,
                            bias=bias_t[:, mm:mm + 1], scale=S_SCALE,
                            accum_out=sslots[:, mm, u:u + 1])
                    else:
                        rr = rscr_p.tile([128, 1024], BF16, tag="rr")
                        nc.vector._custom_dve(
                            _CRA_OP, out=rr[:, :nco], in0=ps[:, :nco],
                            s0=1.0, s1=CLAMP, imm2=_CRA_SEED,
                            accum_out=islots[:, mm, u:u + 1])

            # ---- pack per-core partials and DMA out; host merges ----
            pack = mrg_p.tile([128, 9], F32)
            nc.vector.tensor_copy(out=pack[:, 0:MT], in_=bias_t)
            for m in range(MT):
                nc.vector.reduce_sum(pack[:, MT + m:MT + m + 1],
                                     sslots[:, m, :], axis=AX.X)
            iview = islots[:, :, :].rearrange("p m c -> p (m c)")
            nc.vector.reduce_sum(pack[:, 8:9], iview, axis=AX.X)
            nc.sync.dma_start(out=out_p[:, :], in_=pack[:, :])

    nc.compile()
    return nc


@lru_cache(maxsize=2)
def _graph_cached():
    return _build_graph()


def _host_prep(emb, W, y):
    emb = np.ascontiguousarray(np.asarray(emb), dtype=np.float32)
    W = np.ascontiguousarray(np.asarray(W), dtype=np.float32)
    y = np.asarray(y).astype(np.int64)

    norms = np.sqrt(np.einsum("cd,cd->c", W, W, dtype=np.float64))
    What = (W / norms[:, None].astype(np.float32)).astype(np.float32)
    What8 = What.astype(FP8NP)                      # (C, D) fp8
    emb8 = emb.astype(FP8NP)                        # (B, D) fp8
    ws8 = What8[y]                                  # (B, D) fp8

    def _p_kc(xT):      # (D, ncol) -> (128, KB*ncol) SBUF layout
        return np.ascontiguousarray(
            xT.reshape(KB, 128, -1).transpose(1, 0, 2).reshape(128, -1))

    embT8 = _p_kc(emb8.T)
    wsT8 = _p_kc(ws8.T)

    in_maps = []
    for c in range(NCORES):
        wt_c = np.ascontiguousarray(What8[c * CSH:(c + 1) * CSH].T)
        # unit-major blocks, each [128, KB*w] contiguous per partition
        blk = wt_c.reshape(KB, 128, CSH)
        host = np.concatenate(
            [np.ascontiguousarray(blk[:, :, c0:c0 + w].transpose(1, 0, 2)
                                  ).reshape(128, KB * w)
             for c0, w in UNITS], axis=1)
        in_maps.append({"wt": host, "embt": embT8, "wst": wsT8})
    return in_maps, emb, What, What8, emb8, ws8, y


def _host_merge(packs, emb, What, What8, emb8, ws8, y):
    """Cross-core merge in f64: logsumexp for CE, corrected sum for inter."""
    # pack[p, 0:4]=bias, [4:8]=expsum (row index = m*128+p), [8]=inter
    bias = np.stack([p[:, 0:MT].T.reshape(B) for p in packs])    # (8, B)
    ssum = np.stack([p[:, MT:2 * MT].T.reshape(B) for p in packs])
    nb = -bias.astype(np.float64)           # sum_j e^{l_ij} = s_ic * e^{-b_ic}
    s64 = np.maximum(ssum.astype(np.float64), 1e-300)
    mx = nb.max(axis=0)
    stot = (s64 * np.exp(nb - mx[None, :])).sum(axis=0)
    lse = np.log(stot) + mx                                      # (B,)

    # exact target logit in f64 from the f32-normalized weights
    cos_y = np.einsum("bd,bd->b", emb.astype(np.float64),
                      What[y].astype(np.float64))
    tgt = S_SCALE * (cos_y - MARGIN)
    ce = float(np.mean(lse - tgt))

    inter_raw = float(sum(float(p[:, 8].sum()) for p in packs))
    # subtract the self-term (j == y_i) residues: x = ||w_hat_fp8||^2 - 1
    # clamps to exactly s1 on device; the emulation applies the same min
    n2 = np.einsum("bd,bd->b", ws8.astype(np.float32),
                   ws8.astype(np.float32))
    inter_raw -= float(
        _cra_emulate(n2, 1.0, CLAMP, _CRA_SEED).astype(np.float64).sum())
    inter = -0.5 * inter_raw / (B * (C - 1.0))

    return np.float32(ce + LMD * inter)


def run(emb, W, y, trace=False):
    from concourse.bass_utils import run_bass_kernel_spmd

    in_maps, emb_f, What, What8, emb8, ws8, y64 = _host_prep(emb, W, y)
    nc = _graph_cached()
    res = run_bass_kernel_spmd(nc, in_maps, core_ids=list(range(NCORES)),
                               trace=trace)
    packs = [np.asarray(res.results[c]["out"], dtype=np.float32)
             for c in range(NCORES)]
    val = _host_merge(packs, emb_f, What, What8, emb8, ws8, y64)
    return val, res


def kernel(emb, W, y):
    val, _ = run(emb, W, y, trace=False)
    return val


if __name__ == "__main__":
    rng = np.random.default_rng(0)
    emb = rng.standard_normal((B, D)).astype(np.float32)
    W = rng.standard_normal((C, D)).astype(np.float32)
    y = rng.integers(0, C, size=(B,)).astype(np.int64)
    print("loss:", kernel(emb, W, y))
